# revision 25
# baseline (speedup 1.0000x reference)
"""Multi-head attention (B=2, N=2048, C=1024, H=16, D=64) on 8 TRN2 NeuronCores.

Sharding: core = b*4 + g  (b in {0,1} data parallel over batch,
g in {0..3} tensor parallel over head groups of HL=4 heads).

v3 schedule, built around two measured hardware facts: (1) the ScalarE exp
stream (~163us) is a hard per-core floor, and (2) each matmul costs
LDWEIGHTS + N/2.4GHz serialized (~380ns at N=512) unless adjacent matmuls
sit on disjoint PE row groups, in which case they overlap.

  - A-phase (S^T = K^T.T @ Q^T, K=64): the nh0 half reads the primary
    qt/kt (rows pb..pb+64) and the nh1 half reads the partition-swapped
    copies qt2/kt2 (rows pb^64..), so the four matmuls of every chunk
    alternate PE row groups and run pairwise-concurrent.
  - everything is bf16 on the PE (fast weight load; psum accumulate f32).
  - lead-in: chunked-xt DMA races the QK-mo0 matmuls, first exp ~25us.
  - V is computed inside head 0's loop (no bias matmuls: bias comes from a
    GpSimd partition_broadcast of bv + a tensor_tensor add on the copy out
    of psum); QK-mo1 is split into 2-kc pieces across heads 1-2.
  - per-head norm: one [65,512] copy frees each PSUM accumulator, then
    reciprocal + GpSimd partition_broadcast + DVE mul off-band.
  - projection tail: f32->bf16 casts split across DVE+ScalarE, bf16 DMA.
Host: out[b] = sum_g P^T[b,g].T + proj_b  (bf16 partials summed in f32).
"""

import numpy as np
import ml_dtypes

B, N, C = 2, 2048, 1024
H = 16
D = C // H          # 64
G = 4               # head groups (tensor parallel)
HL = H // G         # 4 heads per core
DL = HL * D         # 256 local head dims
N_CORES = 8
SCALE = 1.0 / np.sqrt(np.float32(D))

MCHUNKS = N // 128  # 16
CO = C // 128       # 8 chunks of the contraction dim c
MO = DL // 128      # 2 chunks of the local head dims

_CACHE = {}
DEBUG_TAPS = False


def build_kernel():
    import concourse.bass as bass
    import concourse.mybir as mybir
    import concourse.tile as tile
    from concourse import bacc

    f32 = mybir.dt.float32
    bf16 = mybir.dt.bfloat16

    nc = bacc.Bacc("TRN2", target_bir_lowering=False, debug=False,
                   num_devices=N_CORES)

    xt_d = nc.dram_tensor("xt", [C, N], bf16, kind="ExternalInput").ap()
    wqt_d = nc.dram_tensor("wqt", [C, DL], bf16, kind="ExternalInput").ap()
    wkt_d = nc.dram_tensor("wkt", [C, DL], bf16, kind="ExternalInput").ap()
    wvt_d = nc.dram_tensor("wvt", [C, DL], bf16, kind="ExternalInput").ap()
    bq_d = nc.dram_tensor("bq", [128, MO], f32, kind="ExternalInput").ap()
    bk_d = nc.dram_tensor("bk", [128, MO], f32, kind="ExternalInput").ap()
    bv_d = nc.dram_tensor("bv", [1, DL], f32, kind="ExternalInput").ap()
    pwt_d = nc.dram_tensor("pwt", [DL, C], bf16, kind="ExternalInput").ap()
    out_d = nc.dram_tensor("out", [C, N], bf16, kind="ExternalOutput").ap()
    if DEBUG_TAPS:
        dbg = {k: nc.dram_tensor(k, shp, dt, kind="ExternalOutput").ap()
               for k, shp, dt in (
                   ("dbg_qt", [128, MO, N], bf16),
                   ("dbg_kt", [128, MO, N], bf16),
                   ("dbg_qt2", [128, MO, N], bf16),
                   ("dbg_kt2", [128, MO, N], bf16),
                   ("dbg_v", [128, MCHUNKS, HL, D + 1], bf16),
                   ("dbg_e00", [128, N], bf16),
                   ("dbg_ob0", [D + 1, 4, 512], f32),
                   ("dbg_rc0", [1, 4, 512], f32),
                   ("dbg_bc0", [D, 4, 512], f32),
                   ("dbg_yt", [128, MO, N], bf16),
               )}

    with tile.TileContext(nc) as tc:
        with (
            tc.tile_pool(name="consts", bufs=1) as consts,
            tc.tile_pool(name="acts", bufs=1) as acts,
            tc.tile_pool(name="xtp", bufs=1) as xtp,
            tc.tile_pool(name="small", bufs=4) as small,
            tc.tile_pool(name="stp", bufs=3) as stp,
            tc.tile_pool(name="eip", bufs=4) as ei_pool,
            tc.tile_pool(name="psS", bufs=2, space="PSUM") as psS,
            tc.tile_pool(name="psB", bufs=4, space="PSUM") as psB,
        ):
            # ---- exp table preload (runs during the input DMAs) ----
            dmy = consts.tile([1, 8], f32, tag="dmy")
            nc.vector.memset(dmy[:], 0.0)
            dmy2 = consts.tile([1, 8], f32, tag="dmy2")
            nc.scalar.activation(dmy2[:], dmy[:],
                                 mybir.ActivationFunctionType.Exp)

            # ---- input DMAs: q/k weights first, then chunked xt ----
            wq_sb = consts.tile([128, CO, DL], bf16, tag="wq")
            wk_sb = consts.tile([128, CO, DL], bf16, tag="wk")
            wv_sb = consts.tile([128, CO, DL], bf16, tag="wv")
            nc.sync.dma_start(wq_sb[:], wqt_d.rearrange("(o p) f -> p o f", p=128))
            nc.sync.dma_start(wk_sb[:], wkt_d.rearrange("(o p) f -> p o f", p=128))
            bq_sb = consts.tile([128, MO], f32, tag="bq")
            bk_sb = consts.tile([128, MO], f32, tag="bk")

            xt_sb = xtp.tile([128, CO, N], bf16, tag="xt")
            xt_r = xt_d.rearrange("(o p) n -> p o n", p=128)
            for kc in range(CO):
                nc.sync.dma_start(xt_sb[:, kc, :], xt_r[:, kc, :])

            nc.sync.dma_start(bq_sb[:], bq_d[:])
            nc.sync.dma_start(bk_sb[:], bk_d[:])
            nc.sync.dma_start(wv_sb[:], wvt_d.rearrange("(o p) f -> p o f", p=128))
            bv_sb = consts.tile([1, DL], f32, tag="bv")
            nc.sync.dma_start(bv_sb[:], bv_d[:])
            pw_sb = consts.tile([128, MO, C], bf16, tag="pw")
            nc.sync.dma_start(pw_sb[:], pwt_d.rearrange("(o p) f -> p o f", p=128))

            # ---- resident activations ----
            qt_sb = acts.tile([128, MO, N], bf16, tag="qt")    # [DL, N]
            kt_sb = acts.tile([128, MO, N], bf16, tag="kt")
            qt2_sb = acts.tile([128, MO, N], bf16, tag="qt2")  # halves swapped
            kt2_sb = acts.tile([128, MO, N], bf16, tag="kt2")
            v_sb = acts.tile([128, MCHUNKS, HL, D + 1], bf16, tag="v")
            yt_sb = acts.tile([128, MO, N], bf16, tag="yt")

            ones_col = consts.tile([128, 1], f32, tag="onescol")
            nc.vector.memset(ones_col[:], 1.0)
            nc.vector.tensor_copy(
                v_sb[:, :, :, D:],
                ones_col[:].to_broadcast([128, MCHUNKS, HL, 1]))
            # broadcast V bias to all partitions once (GpSimd)
            bvb_sb = consts.tile([128, HL, D], f32, tag="bvb")
            nc.gpsimd.partition_broadcast(bvb_sb[:], bv_sb[:])

            # ---- phase-0 helpers ----
            def emit_qk_mms(w_sb, mo, nh, kc_lo, kc_hi, ps=None):
                if ps is None:
                    ps = psS.tile([128, 1024], f32, tag="pss",
                                  name=f"qk{id(w_sb) % 97}_{mo}_{nh}")
                for kc in range(kc_lo, kc_hi):
                    for half in range(2):
                        nc.tensor.matmul(
                            ps[:, half * 512:(half + 1) * 512],
                            lhsT=w_sb[:, kc, mo * 128:(mo + 1) * 128],
                            rhs=xt_sb[:, kc,
                                      nh * 1024 + half * 512:
                                      nh * 1024 + (half + 1) * 512],
                            start=(kc == 0), stop=(kc == CO - 1),
                        )
                return ps

            def emit_qk_bias(ps, b_sb, o_sb, mo, nh):
                nsl0 = slice(nh * 1024, (nh + 1) * 1024)
                nc.vector.tensor_scalar_add(
                    o_sb[:, mo, nsl0], ps[:], b_sb[:, mo:mo + 1])

            def emit_qk_swap(o_sb, o2_sb, mo, nh):
                nsl0 = slice(nh * 1024, (nh + 1) * 1024)
                nc.vector.tensor_copy(o2_sb[0:64, mo, nsl0],
                                      o_sb[64:128, mo, nsl0])
                nc.vector.tensor_copy(o2_sb[64:128, mo, nsl0],
                                      o_sb[0:64, mo, nsl0])

            # mo1 tiles interleaved into heads 1-2 as half-tile bursts of
            # 8 matmuls (~2.5us, just above the ACT queue depth, so the exp
            # stream barely bubbles; a tile held across chunks would starve
            # the 2-slot psum ring)
            def emit_qk_tile_half(which, nh, hf):
                w_sb, b_sb, o_sb, o2_sb = (
                    (wq_sb, bq_sb, qt_sb, qt2_sb) if which == "q"
                    else (wk_sb, bk_sb, kt_sb, kt2_sb))
                ps = psS.tile([128, 512], f32, tag="pss",
                              name=f"qkh_{which}_{nh}_{hf}")
                for kc in range(CO):
                    nc.tensor.matmul(
                        ps[:],
                        lhsT=w_sb[:, kc, 128:256],
                        rhs=xt_sb[:, kc,
                                  nh * 1024 + hf * 512:
                                  nh * 1024 + hf * 512 + 512],
                        start=(kc == 0), stop=(kc == CO - 1),
                    )
                nsl0 = slice(nh * 1024 + hf * 512, nh * 1024 + hf * 512 + 512)
                nc.vector.tensor_scalar_add(
                    o_sb[:, 1, nsl0], ps[:], b_sb[:, 1:2])
                nc.vector.tensor_copy(o2_sb[0:64, 1, nsl0],
                                      o_sb[64:128, 1, nsl0])
                nc.vector.tensor_copy(o2_sb[64:128, 1, nsl0],
                                      o_sb[0:64, 1, nsl0])

            # ---- A-phase chunk: nh0 on primary rows, nh1 on swapped rows
            # (disjoint PE row groups -> the 4 matmuls run pairwise) ----
            def emit_A(h, i, ei):
                mo = h // 2
                pb = 64 * (h % 2)
                pc = pb ^ 64
                ps0 = psS.tile([128, 1024], f32, tag="pss", name=f"a{h}_{i}_0")
                ps1 = psS.tile([128, 1024], f32, tag="pss", name=f"a{h}_{i}_1")
                for half in range(2):
                    nc.tensor.matmul(
                        ps0[:, half * 512:(half + 1) * 512],
                        lhsT=kt_sb[pb:pb + D, mo, i * 128:(i + 1) * 128],
                        rhs=qt_sb[pb:pb + D, mo,
                                  half * 512:half * 512 + 512],
                        start=True, stop=True,
                    )
                    nc.tensor.matmul(
                        ps1[:, half * 512:(half + 1) * 512],
                        lhsT=kt2_sb[pc:pc + D, mo, i * 128:(i + 1) * 128],
                        rhs=qt2_sb[pc:pc + D, mo,
                                   1024 + half * 512:1024 + half * 512 + 512],
                        start=True, stop=True,
                    )
                nc.scalar.activation(ei[:, 0:1024], ps0[:],
                                     mybir.ActivationFunctionType.Exp)
                nc.scalar.activation(ei[:, 1024:2048], ps1[:],
                                     mybir.ActivationFunctionType.Exp)

            # ---- V chunk (inside head 0's loop; bias via bvb add) ----
            def emit_v_chunk(i):
                ps = psS.tile([128, HL, D], f32, tag="pss", name=f"v{i}")
                for kc in range(CO):
                    nc.tensor.matmul(
                        ps[:],
                        lhsT=xt_sb[:, kc, i * 128:(i + 1) * 128],
                        rhs=wv_sb[:, kc, :],
                        start=(kc == 0), stop=(kc == CO - 1),
                    )
                nc.vector.tensor_add(v_sb[:, i, :, :D], ps[:], bvb_sb[:])

            # ---- per-head norm. At head boundaries the psum accumulators
            # are freed first (obs up front); the final norm interleaves
            # per-nb so the projection can start on the first blocks ----
            def emit_norm(hn, psBs_n, interleaved=False):
                mo_n = hn // 2
                pb_n = 64 * (hn % 2)
                obs = []
                for nb in range(4):
                    ob = small.tile([D + 1, 512], f32, tag="ob",
                                    name=f"ob{hn}_{nb}")
                    if not interleaved:
                        nc.vector.tensor_copy(ob[:], psBs_n[nb][:])
                    obs.append(ob)

                def one(nb):
                    nsl = slice(nb * 512, (nb + 1) * 512)
                    if interleaved:
                        nc.vector.tensor_copy(obs[nb][:], psBs_n[nb][:])
                    if DEBUG_TAPS and hn == 0:
                        nc.sync.dma_start(dbg["dbg_ob0"][:, nb, :], obs[nb][:])
                    dn = small.tile([1, 512], f32, tag="dn",
                                    name=f"dn{hn}_{nb}")
                    nc.vector.tensor_copy(dn[:], obs[nb][D:D + 1, :])
                    rc = small.tile([1, 512], f32, tag="rc",
                                    name=f"rc{hn}_{nb}")
                    nc.vector.reciprocal_approx_fast(rc[:], dn[:])
                    bc = small.tile([D, 512], f32, tag="bc",
                                    name=f"bc{hn}_{nb}")
                    nc.gpsimd.partition_broadcast(bc[:], rc[:])
                    if DEBUG_TAPS and hn == 0:
                        nc.sync.dma_start(dbg["dbg_rc0"][:, nb, :], rc[:])
                        nc.sync.dma_start(dbg["dbg_bc0"][:, nb, :], bc[:])
                    nc.vector.tensor_mul(
                        yt_sb[pb_n:pb_n + D, mo_n, nsl], obs[nb][:D, :], bc[:])

                for nb in range(4):
                    one(nb)

            # ---- attention head loop state ----
            psBs_by_h = {}
            pending = []     # queue of (h, i, ei) awaiting B matmuls

            # ---- lead-in: q/k-nh0 tiles, then chunk 0's nh0 exp as early
            # as possible; q-nh1 tile, chunk 0's nh1 exp; k-nh1 comes as a
            # filler in head 0 chunk 1 (first needed at chunk 8) ----
            ps_q0 = emit_qk_mms(wq_sb, 0, 0, 0, CO)
            ps_k0 = emit_qk_mms(wk_sb, 0, 0, 0, CO)
            emit_qk_bias(ps_q0, bq_sb, qt_sb, 0, 0)
            emit_qk_bias(ps_k0, bk_sb, kt_sb, 0, 0)
            emit_qk_swap(qt_sb, qt2_sb, 0, 0)
            emit_qk_swap(kt_sb, kt2_sb, 0, 0)
            ei0 = ei_pool.tile([128, N], bf16, tag="ei", name="ei0_0")
            ps00 = psS.tile([128, 1024], f32, tag="pss", name="a0_0_0")
            for half in range(2):
                nc.tensor.matmul(
                    ps00[:, half * 512:(half + 1) * 512],
                    lhsT=kt_sb[0:D, 0, 0:128],
                    rhs=qt_sb[0:D, 0, half * 512:half * 512 + 512],
                    start=True, stop=True,
                )
            nc.scalar.activation(ei0[:, 0:1024], ps00[:],
                                 mybir.ActivationFunctionType.Exp)
            ps_q1 = emit_qk_mms(wq_sb, 0, 1, 0, CO)
            emit_qk_bias(ps_q1, bq_sb, qt_sb, 0, 1)
            emit_qk_swap(qt_sb, qt2_sb, 0, 1)
            ps01 = psS.tile([128, 1024], f32, tag="pss", name="a0_0_1")
            for half in range(2):
                nc.tensor.matmul(
                    ps01[:, half * 512:(half + 1) * 512],
                    lhsT=kt2_sb[64:64 + D, 0, 0:128],
                    rhs=qt2_sb[64:64 + D, 0,
                               1024 + half * 512:1024 + half * 512 + 512],
                    start=True, stop=True,
                )
            nc.scalar.activation(ei0[:, 1024:2048], ps01[:],
                                 mybir.ActivationFunctionType.Exp)
            pending.append((0, 0, ei0))
            if DEBUG_TAPS:
                nc.sync.dma_start(dbg["dbg_e00"][:], ei0[:])

            def emit_B(hb, ib, eib):
                if ib == 0:
                    if hb > 0:
                        emit_norm(hb - 1, psBs_by_h.pop(hb - 1))
                    psBs_by_h[hb] = [
                        psB.tile([D + 1, 512], f32, tag="psb",
                                 name=f"psb_{hb}_{nb}")
                        for nb in range(4)]
                for nb in range(4):
                    nc.tensor.matmul(
                        psBs_by_h[hb][nb][:],
                        lhsT=v_sb[:, ib, hb, :],
                        rhs=eib[:, nb * 512:(nb + 1) * 512],
                        start=(ib == 0), stop=(ib == MCHUNKS - 1),
                    )

            # mo1 QK half-tile bursts at (head, chunk): q-nh0/k-nh0/q-nh1 in
            # head 1, k-nh1 in head 2 (first needed at head 2 chunk 8)
            filler = {
                (1, 1): ("q", 0, 0), (1, 3): ("q", 0, 1),
                (1, 5): ("k", 0, 0), (1, 7): ("k", 0, 1),
                (1, 9): ("q", 1, 0), (1, 11): ("q", 1, 1),
                (2, 0): ("k", 1, 0), (2, 2): ("k", 1, 1),
            }

            for h in range(HL):
                for i in range(MCHUNKS):
                    if h == 0 and i == 0:
                        continue    # chunk 0 emitted in the lead-in
                    ei = ei_pool.tile([128, N], bf16, tag="ei")
                    emit_A(h, i, ei)
                    if h == 0:
                        # k-nh1-mo0 tile at chunk 1; V chunks from chunk 2
                        # (one chunk late, plus a catch-up pair)
                        if i == 1:
                            ps_k1 = emit_qk_mms(wk_sb, 0, 1, 0, CO)
                            emit_qk_bias(ps_k1, bk_sb, kt_sb, 0, 1)
                            emit_qk_swap(kt_sb, kt2_sb, 0, 1)
                        elif i == 2:
                            emit_v_chunk(0)
                            emit_v_chunk(1)
                        else:
                            emit_v_chunk(i - 1)
                    elif h == 1 and i == 0:
                        emit_v_chunk(15)
                    if (h, i) in filler:
                        emit_qk_tile_half(*filler[(h, i)])
                    if len(pending) >= 2:
                        emit_B(*pending.pop(0))
                    pending.append((h, i, ei))
            if DEBUG_TAPS:
                nc.sync.dma_start(dbg["dbg_qt"][:], qt_sb[:])
                nc.sync.dma_start(dbg["dbg_kt"][:], kt_sb[:])
                nc.sync.dma_start(dbg["dbg_qt2"][:], qt2_sb[:])
                nc.sync.dma_start(dbg["dbg_kt2"][:], kt2_sb[:])
                nc.sync.dma_start(dbg["dbg_v"][:], v_sb[:])
            for p in pending:
                emit_B(*p)
            emit_norm(HL - 1, psBs_by_h.pop(HL - 1), interleaved=True)
            if DEBUG_TAPS:
                nc.sync.dma_start(dbg["dbg_yt"][:], yt_sb[:])

            # ---- phase D: P^T = pwT.T @ Y^T, tail; bf16 out ----
            for nbp in range(2):
                for cc in range(CO):
                    ps = psS.tile([128, 1024], f32, tag="pss",
                                  name=f"d_{nbp}_{cc}")
                    for jc in range(MO):    # jc outer: banks alternate
                        for j in range(2):
                            nb = 2 * nbp + j
                            nc.tensor.matmul(
                                ps[:, j * 512:(j + 1) * 512],
                                lhsT=pw_sb[:, jc, cc * 128:(cc + 1) * 128],
                                rhs=yt_sb[:, jc, nb * 512:(nb + 1) * 512],
                                start=(jc == 0), stop=(jc == MO - 1),
                            )
                    st = stp.tile([128, 1024], bf16, tag="st")
                    nc.vector.tensor_copy(st[:, :512], ps[:, :512])
                    nc.scalar.copy(st[:, 512:], ps[:, 512:])
                    nc.sync.dma_start(
                        out_d[cc * 128:(cc + 1) * 128,
                              nbp * 1024:(nbp + 1) * 1024], st[:])

    nc.compile()
    return nc


def shard_inputs(x, qkv_w, qkv_b, proj_w):
    """Build the 8 per-core input maps (host-side sharding)."""
    in_maps = []
    for core in range(N_CORES):
        b, g = divmod(core, G)
        gs = slice(g * DL, (g + 1) * DL)
        xt = np.ascontiguousarray(x[b].T)
        wq = qkv_w[0 * C:1 * C][gs] * SCALE     # fold 1/sqrt(D) into Q
        wk = qkv_w[1 * C:2 * C][gs]
        wv = qkv_w[2 * C:3 * C][gs]
        in_maps.append({
            "xt": np.ascontiguousarray(xt).astype(ml_dtypes.bfloat16),
            "wqt": np.ascontiguousarray(wq.T).astype(ml_dtypes.bfloat16),
            "wkt": np.ascontiguousarray(wk.T).astype(ml_dtypes.bfloat16),
            "wvt": np.ascontiguousarray(wv.T).astype(ml_dtypes.bfloat16),
            "bq": np.ascontiguousarray(
                (qkv_b[0 * C:1 * C][gs] * SCALE).reshape(DL // 128, 128).T),
            "bk": np.ascontiguousarray(
                qkv_b[1 * C:2 * C][gs].reshape(DL // 128, 128).T),
            "bv": np.ascontiguousarray(qkv_b[2 * C:3 * C][gs].reshape(1, DL)),
            "pwt": np.ascontiguousarray(proj_w[:, gs].T).astype(
                ml_dtypes.bfloat16),
        })
    return in_maps


def unshard_output(results, proj_b):
    """results: list of 8 dicts with 'out' [C, N] bf16 partial projections."""
    out = np.empty((B, N, C), dtype=np.float32)
    for b in range(B):
        acc = results[b * G]["out"].astype(np.float32)
        for g in range(1, G):
            acc = acc + results[b * G + g]["out"].astype(np.float32)
        out[b] = acc.T + proj_b
    return out


def kernel(x, qkv_w, qkv_b, proj_w, proj_b):
    from concourse.bass_utils import run_bass_kernel_spmd

    x = np.asarray(x, dtype=np.float32)
    qkv_w = np.asarray(qkv_w, dtype=np.float32)
    qkv_b = np.asarray(qkv_b, dtype=np.float32)
    proj_w = np.asarray(proj_w, dtype=np.float32)
    proj_b = np.asarray(proj_b, dtype=np.float32)

    if "nc" not in _CACHE:
        _CACHE["nc"] = build_kernel()
    nc = _CACHE["nc"]

    in_maps = shard_inputs(x, qkv_w, qkv_b, proj_w)
    res = run_bass_kernel_spmd(nc, in_maps, list(range(N_CORES)))
    return unshard_output(res.results, proj_b)


# revision 29
# speedup vs baseline: 1.2046x; 1.2046x over previous
"""Multi-head attention (B=2, N=2048, C=1024, H=16, D=64) on 8 TRN2 NeuronCores.

Sharding: core = b*4 + g  (b in {0,1} data parallel over batch,
g in {0..3} tensor parallel over head groups of HL=4 heads).

v3 schedule, built around two measured hardware facts: (1) the ScalarE exp
stream (~163us) is a hard per-core floor, and (2) each matmul costs
LDWEIGHTS + N/2.4GHz serialized (~380ns at N=512) unless adjacent matmuls
sit on disjoint PE row groups, in which case they overlap.

  - A-phase (S^T = K^T.T @ Q^T, K=64): the nh0 half reads the primary
    qt/kt (rows pb..pb+64) and the nh1 half reads the partition-swapped
    copies qt2/kt2 (rows pb^64..), so the four matmuls of every chunk
    alternate PE row groups and run pairwise-concurrent.
  - everything is bf16 on the PE (fast weight load; psum accumulate f32).
  - lead-in: chunked-xt DMA races the QK-mo0 matmuls, first exp ~25us.
  - V is computed inside head 0's loop (no bias matmuls: bias comes from a
    GpSimd partition_broadcast of bv + a tensor_tensor add on the copy out
    of psum); QK-mo1 is split into 2-kc pieces across heads 1-2.
  - per-head norm: one [65,512] copy frees each PSUM accumulator, then
    reciprocal + GpSimd partition_broadcast + DVE mul off-band.
  - projection tail: f32->bf16 casts split across DVE+ScalarE, bf16 DMA.
Host: out[b] = sum_g P^T[b,g].T + proj_b  (bf16 partials summed in f32).
"""

import numpy as np
import ml_dtypes

B, N, C = 2, 2048, 1024
H = 16
D = C // H          # 64
G = 4               # head groups (tensor parallel)
HL = H // G         # 4 heads per core
DL = HL * D         # 256 local head dims
N_CORES = 8
SCALE = 1.0 / np.sqrt(np.float32(D))

MCHUNKS = N // 128  # 16
CO = C // 128       # 8 chunks of the contraction dim c
MO = DL // 128      # 2 chunks of the local head dims

_CACHE = {}
DEBUG_TAPS = False


def build_kernel():
    import concourse.bass as bass
    import concourse.mybir as mybir
    import concourse.tile as tile
    from concourse import bacc

    f32 = mybir.dt.float32
    bf16 = mybir.dt.bfloat16

    nc = bacc.Bacc("TRN2", target_bir_lowering=False, debug=False,
                   num_devices=N_CORES)

    xt_d = nc.dram_tensor("xt", [C, N], bf16, kind="ExternalInput").ap()
    wqt_d = nc.dram_tensor("wqt", [C, DL], bf16, kind="ExternalInput").ap()
    wkt_d = nc.dram_tensor("wkt", [C, DL], bf16, kind="ExternalInput").ap()
    wvt_d = nc.dram_tensor("wvt", [C, DL], bf16, kind="ExternalInput").ap()
    bq_d = nc.dram_tensor("bq", [128, MO], f32, kind="ExternalInput").ap()
    bk_d = nc.dram_tensor("bk", [128, MO], f32, kind="ExternalInput").ap()
    bv_d = nc.dram_tensor("bv", [1, DL], f32, kind="ExternalInput").ap()
    pwt_d = nc.dram_tensor("pwt", [DL, C], bf16, kind="ExternalInput").ap()
    out_d = nc.dram_tensor("out", [C, N], bf16, kind="ExternalOutput").ap()
    if DEBUG_TAPS:
        dbg = {k: nc.dram_tensor(k, shp, dt, kind="ExternalOutput").ap()
               for k, shp, dt in (
                   ("dbg_qt", [128, MO, N], bf16),
                   ("dbg_kt", [128, MO, N], bf16),
                   ("dbg_qt2", [128, MO, N], bf16),
                   ("dbg_kt2", [128, MO, N], bf16),
                   ("dbg_v", [128, MCHUNKS, HL, D + 1], bf16),
                   ("dbg_e00", [128, N], bf16),
                   ("dbg_ob0", [D + 1, 4, 512], f32),
                   ("dbg_rc0", [1, 4, 512], f32),
                   ("dbg_bc0", [D, 4, 512], f32),
                   ("dbg_yt", [128, MO, N], bf16),
               )}

    with tile.TileContext(nc) as tc:
        with (
            tc.tile_pool(name="consts", bufs=1) as consts,
            tc.tile_pool(name="acts", bufs=1) as acts,
            tc.tile_pool(name="xtp", bufs=1) as xtp,
            tc.tile_pool(name="small", bufs=4) as small,
            tc.tile_pool(name="stp", bufs=3) as stp,
            tc.tile_pool(name="eip", bufs=4) as ei_pool,
            tc.tile_pool(name="psS", bufs=2, space="PSUM") as psS,
            tc.tile_pool(name="psB", bufs=4, space="PSUM") as psB,
        ):
            # ---- exp table preload (runs during the input DMAs) ----
            dmy = consts.tile([1, 8], f32, tag="dmy")
            nc.vector.memset(dmy[:], 0.0)
            dmy2 = consts.tile([1, 8], f32, tag="dmy2")
            nc.scalar.activation(dmy2[:], dmy[:],
                                 mybir.ActivationFunctionType.Exp)

            # ---- input DMAs: q/k weights first, then chunked xt ----
            wq_sb = consts.tile([128, CO, DL], bf16, tag="wq")
            wk_sb = consts.tile([128, CO, DL], bf16, tag="wk")
            wv_sb = consts.tile([128, CO, DL], bf16, tag="wv")
            nc.sync.dma_start(wq_sb[:], wqt_d.rearrange("(o p) f -> p o f", p=128))
            nc.sync.dma_start(wk_sb[:], wkt_d.rearrange("(o p) f -> p o f", p=128))
            bq_sb = consts.tile([128, MO], f32, tag="bq")
            bk_sb = consts.tile([128, MO], f32, tag="bk")

            xt_sb = xtp.tile([128, CO, N], bf16, tag="xt")
            xt_r = xt_d.rearrange("(o p) n -> p o n", p=128)
            for kc in range(CO):
                nc.sync.dma_start(xt_sb[:, kc, :], xt_r[:, kc, :])

            nc.sync.dma_start(bq_sb[:], bq_d[:])
            nc.sync.dma_start(bk_sb[:], bk_d[:])
            nc.sync.dma_start(wv_sb[:], wvt_d.rearrange("(o p) f -> p o f", p=128))
            bv_sb = consts.tile([1, DL], f32, tag="bv")
            nc.sync.dma_start(bv_sb[:], bv_d[:])
            pw_sb = consts.tile([128, MO, C], bf16, tag="pw")
            nc.sync.dma_start(pw_sb[:], pwt_d.rearrange("(o p) f -> p o f", p=128))

            # ---- resident activations ----
            qt_sb = acts.tile([128, MO, N], bf16, tag="qt")    # [DL, N]
            kt_sb = acts.tile([128, MO, N], bf16, tag="kt")
            qt2_sb = acts.tile([128, MO, N], bf16, tag="qt2")  # halves swapped
            kt2_sb = acts.tile([128, MO, N], bf16, tag="kt2")
            v_sb = acts.tile([128, MCHUNKS, HL, D + 1], bf16, tag="v")
            yt_sb = acts.tile([128, MO, N], bf16, tag="yt")

            ones_col = consts.tile([128, 1], f32, tag="onescol")
            nc.vector.memset(ones_col[:], 1.0)
            nc.vector.tensor_copy(
                v_sb[:, :, :, D:],
                ones_col[:].to_broadcast([128, MCHUNKS, HL, 1]))
            # broadcast V bias to all partitions once (GpSimd)
            bvb_sb = consts.tile([128, HL, D], f32, tag="bvb")
            nc.gpsimd.partition_broadcast(bvb_sb[:], bv_sb[:])

            # ---- phase-0 helpers ----
            def emit_qk_mms(w_sb, mo, nh, kc_lo, kc_hi, ps=None):
                if ps is None:
                    ps = psS.tile([128, 1024], f32, tag="pss",
                                  name=f"qk{id(w_sb) % 97}_{mo}_{nh}")
                for kc in range(kc_lo, kc_hi):
                    for half in range(2):
                        nc.tensor.matmul(
                            ps[:, half * 512:(half + 1) * 512],
                            lhsT=w_sb[:, kc, mo * 128:(mo + 1) * 128],
                            rhs=xt_sb[:, kc,
                                      nh * 1024 + half * 512:
                                      nh * 1024 + (half + 1) * 512],
                            start=(kc == 0), stop=(kc == CO - 1),
                        )
                return ps

            def emit_qk_bias(ps, b_sb, o_sb, mo, nh):
                nsl0 = slice(nh * 1024, (nh + 1) * 1024)
                nc.vector.tensor_scalar_add(
                    o_sb[:, mo, nsl0], ps[:], b_sb[:, mo:mo + 1])

            def emit_qk_swap(o_sb, o2_sb, mo, nh):
                nsl0 = slice(nh * 1024, (nh + 1) * 1024)
                nc.vector.tensor_copy(o2_sb[0:64, mo, nsl0],
                                      o_sb[64:128, mo, nsl0])
                nc.vector.tensor_copy(o2_sb[64:128, mo, nsl0],
                                      o_sb[0:64, mo, nsl0])

            # mo1 tiles interleaved into heads 1-2 as half-tile bursts of
            # 8 matmuls (~2.5us, just above the ACT queue depth, so the exp
            # stream barely bubbles; a tile held across chunks would starve
            # the 2-slot psum ring)
            def emit_qk_tile_half(which, nh, hf):
                w_sb, b_sb, o_sb, o2_sb = (
                    (wq_sb, bq_sb, qt_sb, qt2_sb) if which == "q"
                    else (wk_sb, bk_sb, kt_sb, kt2_sb))
                ps = psS.tile([128, 512], f32, tag="pss",
                              name=f"qkh_{which}_{nh}_{hf}")
                for kc in range(CO):
                    nc.tensor.matmul(
                        ps[:],
                        lhsT=w_sb[:, kc, 128:256],
                        rhs=xt_sb[:, kc,
                                  nh * 1024 + hf * 512:
                                  nh * 1024 + hf * 512 + 512],
                        start=(kc == 0), stop=(kc == CO - 1),
                    )
                nsl0 = slice(nh * 1024 + hf * 512, nh * 1024 + hf * 512 + 512)
                nc.vector.tensor_scalar_add(
                    o_sb[:, 1, nsl0], ps[:], b_sb[:, 1:2])
                nc.vector.tensor_copy(o2_sb[0:64, 1, nsl0],
                                      o_sb[64:128, 1, nsl0])
                nc.vector.tensor_copy(o2_sb[64:128, 1, nsl0],
                                      o_sb[0:64, 1, nsl0])

            # ---- A-phase chunk: nh0 on primary rows, nh1 on swapped rows
            # (disjoint PE row groups -> the 4 matmuls run pairwise) ----
            def emit_A(h, i, ei):
                mo = h // 2
                pb = 64 * (h % 2)
                pc = pb ^ 64
                ps0 = psS.tile([128, 1024], f32, tag="pss", name=f"a{h}_{i}_0")
                ps1 = psS.tile([128, 1024], f32, tag="pss", name=f"a{h}_{i}_1")
                for half in range(2):
                    nc.tensor.matmul(
                        ps0[:, half * 512:(half + 1) * 512],
                        lhsT=kt_sb[pb:pb + D, mo, i * 128:(i + 1) * 128],
                        rhs=qt_sb[pb:pb + D, mo,
                                  half * 512:half * 512 + 512],
                        start=True, stop=True,
                    )
                    nc.tensor.matmul(
                        ps1[:, half * 512:(half + 1) * 512],
                        lhsT=kt2_sb[pc:pc + D, mo, i * 128:(i + 1) * 128],
                        rhs=qt2_sb[pc:pc + D, mo,
                                   1024 + half * 512:1024 + half * 512 + 512],
                        start=True, stop=True,
                    )
                nc.scalar.activation(ei[:, 0:1024], ps0[:],
                                     mybir.ActivationFunctionType.Exp)
                nc.scalar.activation(ei[:, 1024:2048], ps1[:],
                                     mybir.ActivationFunctionType.Exp)

            # ---- V chunk (inside head 0's loop; bias via bvb add) ----
            def emit_v_chunk(i):
                ps = psS.tile([128, HL, D], f32, tag="pss", name=f"v{i}")
                for kc in range(CO):
                    nc.tensor.matmul(
                        ps[:],
                        lhsT=xt_sb[:, kc, i * 128:(i + 1) * 128],
                        rhs=wv_sb[:, kc, :],
                        start=(kc == 0), stop=(kc == CO - 1),
                    )
                nc.vector.tensor_add(v_sb[:, i, :, :D], ps[:], bvb_sb[:])

            # ---- per-head norm. At head boundaries the psum accumulators
            # are freed first (obs up front); the final norm interleaves
            # per-nb so the projection can start on the first blocks ----
            def emit_norm(hn, psBs_n, interleaved=False):
                mo_n = hn // 2
                pb_n = 64 * (hn % 2)
                obs = []
                for nb in range(4):
                    ob = small.tile([D + 1, 512], f32, tag="ob",
                                    name=f"ob{hn}_{nb}")
                    if not interleaved:
                        nc.vector.tensor_copy(ob[:], psBs_n[nb][:])
                    obs.append(ob)

                def one(nb):
                    nsl = slice(nb * 512, (nb + 1) * 512)
                    if interleaved:
                        nc.vector.tensor_copy(obs[nb][:], psBs_n[nb][:])
                    if DEBUG_TAPS and hn == 0:
                        nc.sync.dma_start(dbg["dbg_ob0"][:, nb, :], obs[nb][:])
                    dn = small.tile([1, 512], f32, tag="dn",
                                    name=f"dn{hn}_{nb}")
                    nc.vector.tensor_copy(dn[:], obs[nb][D:D + 1, :])
                    rc = small.tile([1, 512], f32, tag="rc",
                                    name=f"rc{hn}_{nb}")
                    nc.vector.reciprocal_approx_fast(rc[:], dn[:])
                    bc = small.tile([D, 512], f32, tag="bc",
                                    name=f"bc{hn}_{nb}")
                    nc.gpsimd.partition_broadcast(bc[:], rc[:])
                    if DEBUG_TAPS and hn == 0:
                        nc.sync.dma_start(dbg["dbg_rc0"][:, nb, :], rc[:])
                        nc.sync.dma_start(dbg["dbg_bc0"][:, nb, :], bc[:])
                    nc.vector.tensor_mul(
                        yt_sb[pb_n:pb_n + D, mo_n, nsl], obs[nb][:D, :], bc[:])

                for nb in range(4):
                    one(nb)

            # ---- attention head loop state ----
            psBs_by_h = {}
            pending = []     # queue of (h, i, ei) awaiting B matmuls

            # ---- lead-in: q/k-nh0 tiles, then chunk 0's nh0 exp as early
            # as possible; q-nh1 tile, chunk 0's nh1 exp; k-nh1 comes as a
            # filler in head 0 chunk 1 (first needed at chunk 8) ----
            ps_q0 = emit_qk_mms(wq_sb, 0, 0, 0, CO)
            ps_k0 = emit_qk_mms(wk_sb, 0, 0, 0, CO)
            emit_qk_bias(ps_q0, bq_sb, qt_sb, 0, 0)
            emit_qk_bias(ps_k0, bk_sb, kt_sb, 0, 0)
            emit_qk_swap(qt_sb, qt2_sb, 0, 0)
            emit_qk_swap(kt_sb, kt2_sb, 0, 0)
            ei0 = ei_pool.tile([128, N], bf16, tag="ei", name="ei0_0")
            ps00 = psS.tile([128, 1024], f32, tag="pss", name="a0_0_0")
            for half in range(2):
                nc.tensor.matmul(
                    ps00[:, half * 512:(half + 1) * 512],
                    lhsT=kt_sb[0:D, 0, 0:128],
                    rhs=qt_sb[0:D, 0, half * 512:half * 512 + 512],
                    start=True, stop=True,
                )
            nc.scalar.activation(ei0[:, 0:1024], ps00[:],
                                 mybir.ActivationFunctionType.Exp)
            ps_q1 = emit_qk_mms(wq_sb, 0, 1, 0, CO)
            emit_qk_bias(ps_q1, bq_sb, qt_sb, 0, 1)
            emit_qk_swap(qt_sb, qt2_sb, 0, 1)
            ps01 = psS.tile([128, 1024], f32, tag="pss", name="a0_0_1")
            for half in range(2):
                nc.tensor.matmul(
                    ps01[:, half * 512:(half + 1) * 512],
                    lhsT=kt2_sb[64:64 + D, 0, 0:128],
                    rhs=qt2_sb[64:64 + D, 0,
                               1024 + half * 512:1024 + half * 512 + 512],
                    start=True, stop=True,
                )
            nc.scalar.activation(ei0[:, 1024:2048], ps01[:],
                                 mybir.ActivationFunctionType.Exp)
            pending.append((0, 0, ei0))
            if DEBUG_TAPS:
                nc.sync.dma_start(dbg["dbg_e00"][:], ei0[:])
            emit_v_chunk(0)
            ps_k1 = emit_qk_mms(wk_sb, 0, 1, 0, CO)
            emit_qk_bias(ps_k1, bk_sb, kt_sb, 0, 1)
            emit_qk_swap(kt_sb, kt2_sb, 0, 1)

            def emit_B(hb, ib, eib):
                if ib == 0:
                    if hb > 0:
                        emit_norm(hb - 1, psBs_by_h.pop(hb - 1))
                    psBs_by_h[hb] = [
                        psB.tile([D + 1, 512], f32, tag="psb",
                                 name=f"psb_{hb}_{nb}")
                        for nb in range(4)]
                for nb in range(4):
                    nc.tensor.matmul(
                        psBs_by_h[hb][nb][:],
                        lhsT=v_sb[:, ib, hb, :],
                        rhs=eib[:, nb * 512:(nb + 1) * 512],
                        start=(ib == 0), stop=(ib == MCHUNKS - 1),
                    )

            # mo1 QK half-tile bursts at (head, chunk): q-nh0/k-nh0/q-nh1 in
            # head 1, k-nh1 in head 2 (first needed at head 2 chunk 8)
            filler = {
                (1, 1): ("q", 0, 0), (1, 3): ("q", 0, 1),
                (1, 6): ("k", 0, 0), (1, 8): ("k", 0, 1),
                (1, 11): ("q", 1, 0), (1, 13): ("q", 1, 1),
                (2, 0): ("k", 1, 0), (2, 2): ("k", 1, 1),
            }

            for h in range(HL):
                for i in range(MCHUNKS):
                    if h == 0 and i == 0:
                        continue    # chunk 0 emitted in the lead-in
                    ei = ei_pool.tile([128, N], bf16, tag="ei")
                    emit_A(h, i, ei)
                    if h == 0:
                        emit_v_chunk(i)
                    if (h, i) in filler:
                        emit_qk_tile_half(*filler[(h, i)])
                    if len(pending) >= 2:
                        emit_B(*pending.pop(0))
                    pending.append((h, i, ei))
            if DEBUG_TAPS:
                nc.sync.dma_start(dbg["dbg_qt"][:], qt_sb[:])
                nc.sync.dma_start(dbg["dbg_kt"][:], kt_sb[:])
                nc.sync.dma_start(dbg["dbg_qt2"][:], qt2_sb[:])
                nc.sync.dma_start(dbg["dbg_kt2"][:], kt2_sb[:])
                nc.sync.dma_start(dbg["dbg_v"][:], v_sb[:])
            for p in pending:
                emit_B(*p)
            emit_norm(HL - 1, psBs_by_h.pop(HL - 1), interleaved=True)
            if DEBUG_TAPS:
                nc.sync.dma_start(dbg["dbg_yt"][:], yt_sb[:])

            # ---- phase D: P^T = pwT.T @ Y^T, tail; bf16 out ----
            for nbp in range(2):
                for cc in range(CO):
                    ps = psS.tile([128, 1024], f32, tag="pss",
                                  name=f"d_{nbp}_{cc}")
                    for jc in range(MO):    # jc outer: banks alternate
                        for j in range(2):
                            nb = 2 * nbp + j
                            nc.tensor.matmul(
                                ps[:, j * 512:(j + 1) * 512],
                                lhsT=pw_sb[:, jc, cc * 128:(cc + 1) * 128],
                                rhs=yt_sb[:, jc, nb * 512:(nb + 1) * 512],
                                start=(jc == 0), stop=(jc == MO - 1),
                            )
                    st = stp.tile([128, 1024], bf16, tag="st")
                    nc.vector.tensor_copy(st[:, :512], ps[:, :512])
                    nc.scalar.copy(st[:, 512:], ps[:, 512:])
                    nc.sync.dma_start(
                        out_d[cc * 128:(cc + 1) * 128,
                              nbp * 1024:(nbp + 1) * 1024], st[:])

    nc.compile()
    return nc


def shard_inputs(x, qkv_w, qkv_b, proj_w):
    """Build the 8 per-core input maps (host-side sharding)."""
    in_maps = []
    for core in range(N_CORES):
        b, g = divmod(core, G)
        gs = slice(g * DL, (g + 1) * DL)
        xt = np.ascontiguousarray(x[b].T)
        wq = qkv_w[0 * C:1 * C][gs] * SCALE     # fold 1/sqrt(D) into Q
        wk = qkv_w[1 * C:2 * C][gs]
        wv = qkv_w[2 * C:3 * C][gs]
        in_maps.append({
            "xt": np.ascontiguousarray(xt).astype(ml_dtypes.bfloat16),
            "wqt": np.ascontiguousarray(wq.T).astype(ml_dtypes.bfloat16),
            "wkt": np.ascontiguousarray(wk.T).astype(ml_dtypes.bfloat16),
            "wvt": np.ascontiguousarray(wv.T).astype(ml_dtypes.bfloat16),
            "bq": np.ascontiguousarray(
                (qkv_b[0 * C:1 * C][gs] * SCALE).reshape(DL // 128, 128).T),
            "bk": np.ascontiguousarray(
                qkv_b[1 * C:2 * C][gs].reshape(DL // 128, 128).T),
            "bv": np.ascontiguousarray(qkv_b[2 * C:3 * C][gs].reshape(1, DL)),
            "pwt": np.ascontiguousarray(proj_w[:, gs].T).astype(
                ml_dtypes.bfloat16),
        })
    return in_maps


def unshard_output(results, proj_b):
    """results: list of 8 dicts with 'out' [C, N] bf16 partial projections."""
    out = np.empty((B, N, C), dtype=np.float32)
    for b in range(B):
        acc = results[b * G]["out"].astype(np.float32)
        for g in range(1, G):
            acc = acc + results[b * G + g]["out"].astype(np.float32)
        out[b] = acc.T + proj_b
    return out


def kernel(x, qkv_w, qkv_b, proj_w, proj_b):
    from concourse.bass_utils import run_bass_kernel_spmd

    x = np.asarray(x, dtype=np.float32)
    qkv_w = np.asarray(qkv_w, dtype=np.float32)
    qkv_b = np.asarray(qkv_b, dtype=np.float32)
    proj_w = np.asarray(proj_w, dtype=np.float32)
    proj_b = np.asarray(proj_b, dtype=np.float32)

    if "nc" not in _CACHE:
        _CACHE["nc"] = build_kernel()
    nc = _CACHE["nc"]

    in_maps = shard_inputs(x, qkv_w, qkv_b, proj_w)
    res = run_bass_kernel_spmd(nc, in_maps, list(range(N_CORES)))
    return unshard_output(res.results, proj_b)
